# revision 37
# baseline (speedup 1.0000x reference)
"""Trainium2 Bass kernel for nn_LocalModel_Layer_35493609734520.

out[n] = sum_d x[n, d] * W[idx[n], d]   (gather row of W, dot with x row)

Sort-based formulation, data-parallel over rows across 8 cores:
  - The host stably sorts rows by idx and pads each class segment to a
    multiple of 128 rows, so every 128-row tile has a single class c and
    out[j] = sum_d W[c, d] * xT[d, j] for the whole tile.  The padded
    stream is 288 tiles (36864 rows) per core -- enough for any input
    (worst-case padding is 127 rows per class).  The permutation is
    inverted on the host after the run; padded rows are discarded.
  - The host ships xT (sorted x, transposed, bf16) and per-tile weight
    columns wsegT (bf16).  bf16 halves the HBM stream (~3e-3 rel err
    against a 2e-2 gate).
  - On device each tile is two K=128, M=1 matmuls on TensorE:
    out_row[1, 128] += wcol[128, 1]^T @ xT_half[128, 128], accumulated
    over the two d-halves into one PSUM partition.  128 tiles fill one
    [128, 128] PSUM group, ScalarE copies it to SBUF, and a 512B-chunk
    DMA stores it.  TensorE cost is 128 cycles per matmul regardless of
    M, so the whole gather+dot runs at ~31 us/core on PE; the kernel is
    DMA-bound (~52 us of HBM streaming).
"""

import numpy as np

N = 262144
D = 256
C = 256
NCORES = 8
P = 128
T_CAP = 288  # padded tiles per core: 8*288*128 = 294912 >= N + 256*127
NPC = T_CAP * P  # 36864 padded rows per core
CHUNK = 16  # tiles per x-DMA chunk (2048 columns)

_compiled = None


def _build():
    import contextlib

    import concourse.mybir as mybir
    import concourse.tile as tile
    from concourse import bacc

    f32 = mybir.dt.float32
    bf16 = mybir.dt.bfloat16

    nc = bacc.Bacc("TRN2", target_bir_lowering=False, debug=False)

    # xT[h, d, j] = x_sorted[j, h*128 + d]
    xt_d = nc.dram_tensor("xT", [2, P, NPC], bf16, kind="ExternalInput").ap()
    # wsegT[d, 2*t + h] = W[class(t), h*128 + d]
    ws_d = nc.dram_tensor("wsegT", [P, 2 * T_CAP], bf16, kind="ExternalInput").ap()
    out_d = nc.dram_tensor("out", [NPC, 1], f32, kind="ExternalOutput").ap()

    nchunks = T_CAP // CHUNK
    CW = CHUNK * P  # 2048 output values per chunk
    HW = CW // 2  # 1024 values per PSUM strip (half chunk, 8 tiles)

    # xT viewed so one DMA fetches both d-halves of a chunk: [p, h, j]
    xt_view = xt_d.rearrange("h p j -> p h j")

    with tile.TileContext(nc) as tc:
        with contextlib.ExitStack() as ctx:
            const = ctx.enter_context(tc.tile_pool(name="const", bufs=1))
            xpool = ctx.enter_context(tc.tile_pool(name="xp", bufs=4))
            gpool = ctx.enter_context(tc.tile_pool(name="po", bufs=4, space="PSUM"))
            opool = ctx.enter_context(tc.tile_pool(name="op", bufs=3))

            wt = const.tile([P, 2 * T_CAP], bf16, tag="wt")
            nc.sync.dma_start(wt[:], ws_d)

            for ci in range(nchunks):
                xt = xpool.tile([P, 2, CW], bf16, tag="xt")
                nc.sync.dma_start(xt[:], xt_view[:, :, ci * CW : (ci + 1) * CW])
                osb = opool.tile([1, CW], f32, tag="osb")
                # all tile outputs land along the free axis of
                # single-partition PSUM strips (M=1 matmuls must write base
                # partition 0); half-chunk strips keep a 4-deep rotation
                # within the 16KB single-partition PSUM budget
                for s in range(2):
                    po = gpool.tile([1, HW], f32, tag="po")
                    for k in range(CHUNK // 2):
                        kk = s * (CHUNK // 2) + k
                        t = ci * CHUNK + kk
                        for h in range(2):
                            nc.tensor.matmul(
                                po[:, k * P : (k + 1) * P],
                                wt[:, 2 * t + h : 2 * t + h + 1],
                                xt[:, h, kk * P : (kk + 1) * P],
                                start=(h == 0),
                                stop=(h == 1),
                            )
                    # drains split between ScalarE and the idle VectorE
                    dst = osb[:, s * HW : (s + 1) * HW]
                    if s == 0:
                        nc.scalar.copy(dst, po[:])
                    else:
                        nc.vector.tensor_copy(dst, po[:])
                out_dst = out_d[ci * CW : (ci + 1) * CW, :].rearrange(
                    "(a j) one -> a (j one)", a=1
                )
                # issue from VectorE (it wrote the last half, so the
                # data-ready wait is locally satisfied and never stalls
                # SP's x-DMA issue stream)
                nc.vector.dma_start(out_dst, osb[:])

    nc.compile()
    return nc


def _get_compiled():
    global _compiled
    if _compiled is None:
        _compiled = _build()
    return _compiled


def _pack_inputs(x, idx, W):
    """Sort rows by class, pad class segments to 128-multiples, build
    per-core transposed inputs.  Returns (in_maps, slots, nreal)."""
    import ml_dtypes

    bf16 = ml_dtypes.bfloat16
    n = x.shape[0]
    x_bf = np.asarray(x, dtype=np.float32).astype(bf16)
    idx_flat = np.asarray(idx).reshape(-1).astype(np.int64)
    W_bf = np.asarray(W, dtype=np.float32).astype(bf16)

    order = np.argsort(idx_flat, kind="stable")
    counts = np.bincount(idx_flat, minlength=C)
    padded = (counts + P - 1) // P * P
    starts = np.zeros(C + 1, dtype=np.int64)
    np.cumsum(padded, out=starts[1:])
    total = int(starts[-1])
    cap = NCORES * NPC
    assert total <= cap, f"padded rows {total} exceed capacity {cap}"

    # slot in the padded stream for each row of the sorted order
    unp_starts = np.zeros(C, dtype=np.int64)
    np.cumsum(counts[:-1], out=unp_starts[1:])
    rank = np.arange(n, dtype=np.int64) - np.repeat(unp_starts, counts)
    slots = np.repeat(starts[:-1], counts) + rank

    x_pack = np.zeros((cap, D), dtype=bf16)
    x_pack[slots] = x_bf[order]

    # class of each padded tile (tail tiles -> class 0 with zero rows)
    tile_cls = np.minimum(
        np.searchsorted(starts, np.arange(cap // P) * P, side="right") - 1, C - 1
    )
    W2 = W_bf.reshape(C, 2, P)

    in_maps = []
    for c in range(NCORES):
        xs = x_pack[c * NPC : (c + 1) * NPC]  # [NPC, 256]
        xt = np.ascontiguousarray(xs.T.reshape(2, P, NPC))
        cls = tile_cls[c * T_CAP : (c + 1) * T_CAP]
        # [T_CAP, 2, 128] -> [128 d, T_CAP, 2] -> [128, 2*T_CAP]
        ws = np.ascontiguousarray(
            W2[cls].transpose(2, 0, 1).reshape(P, 2 * T_CAP)
        )
        in_maps.append({"xT": xt, "wsegT": ws})
    return in_maps, slots, order


def kernel(x, idx, W):
    from concourse.bass_utils import run_bass_kernel_spmd

    nc = _get_compiled()
    in_maps, slots, order = _pack_inputs(x, idx, W)
    res = run_bass_kernel_spmd(nc, in_maps, core_ids=list(range(NCORES)))
    out_sorted = np.concatenate(
        [res.results[c]["out"].reshape(-1) for c in range(NCORES)]
    )
    out = np.empty(N, dtype=np.float32)
    out[order] = out_sorted[slots]
    return out.reshape(N, 1)
